# revision 28
# baseline (speedup 1.0000x reference)
"""CRF Viterbi decode kernel for Trainium2 (8 NeuronCores, Bass/Tile).

Problem: B=256, L=512, D=768, C=19 (17 tags + START + STOP).
  emis = features @ W_fc.T + b_fc                        [B, L, C]
  Viterbi forward (max-plus recurrence over L) + backpointers
  -> best_score [B] f32, best_paths [B, L] int32

Sharding: data-parallel over batch B -> 32 sequences per core, batch on
SBUF partitions. Small weights (W_fc, transitions) replicated.

Exactness strategy: the reference's argmax decisions are reproduced
bit-for-bit. The DP is restricted to the 17 "active" tags: the START row
and STOP column of `transitions` are -10000, so START/STOP never win any
argmax after t=0 (scores stay within a few hundred of each other while
the -10000 offset is insurmountable); t=0 is handled in closed form
(s_0 = T[:, START] + emis_0). All forward adds/maxes use the same fp32
operation order as the reference. Backpointers are recovered by an
equality pass against the stored per-step max values (recomputing
prev + T with the identical fp32 add), which reproduces jnp.argmax's
first-max-index semantics exactly, ties included.

Per-core phases (engine):
  E  (DMA/PE/ACT/DVE): stream features, PE-transpose 128x128 blocks,
      matmul vs W (+bias as a K=1 matmul) -> emis, scatter into
      [32b, 32t, 17c] chunk tiles; PE part runs two chunks ahead of the
      forward, PSUM->SBUF copies one chunk ahead
  F  (DVE, serial): 511 x (add T -> segmented max -> add emis), the
      critical path (~500 us of DVE busy in the cost model)
  BP (3 stages trailing F by 1/2/3 chunks): DMA replicates scores 4x
      across partitions + block-broadcasts them; GPSIMD adds T and
      multiplies the iota mask; DVE does the exact is_equal against the
      stored per-step maxes and the first-max-index min-reduce
  BT (host): vectorized numpy backtrack over the downloaded bp table

Under this axon client no NTFF/neuron-profile path exists; per the
TimelineSim instruction cost model one core executes in ~727 us
(DVE-bound), overlapping the ~50 MB/core feature stream, PE matmuls,
GPSIMD bp work and the DMA traffic.
"""

import numpy as np

import concourse.bass as bass
import concourse.mybir as mybir
import concourse.tile as tile
from concourse import bacc
from concourse.bass_utils import run_bass_kernel_spmd
from concourse.masks import make_identity

F32 = mybir.dt.float32
U8 = mybir.dt.uint8

N_CORES = 8
B, L, D, C = 256, 512, 768, 19
CA = 17                      # active tags (START=17, STOP=18 excluded)
CC = CA * CA                 # 289
BSH = B // N_CORES           # 32 sequences per core
R = BSH * L                  # 16384 rows of the emis matmul per core
NCH = L // 32                # 16 chunks of 32 timesteps
START, STOP = C - 2, C - 1
BIG = 1.0e6


def _ap(base, pairs):
    """AP with the partition pair of `base` and explicit free pairs."""
    return bass.AP(tensor=base.tensor, offset=base.offset,
                   ap=[list(base.ap[0])] + [list(p) for p in pairs])


def build_bass(skip_bp=False, skip_emis=False):
    nc = bacc.Bacc("TRN2", target_bir_lowering=False, debug=False,
                   num_devices=N_CORES)

    feat = nc.dram_tensor("feat", [R, D], F32, kind="ExternalInput")
    wt = nc.dram_tensor("wt", [D, CA], F32, kind="ExternalInput")
    trep = nc.dram_tensor("trep", [1, CC], F32, kind="ExternalInput")
    trepx = nc.dram_tensor("trepx", [1, 8 * CC], F32, kind="ExternalInput")
    iotamx = nc.dram_tensor("iotamx", [1, 8 * CC], F32, kind="ExternalInput")
    tcol = nc.dram_tensor("tcol", [1, CA], F32, kind="ExternalInput")
    tstop = nc.dram_tensor("tstop", [1, CA], F32, kind="ExternalInput")
    bfc = nc.dram_tensor("bfc", [1, CA], F32, kind="ExternalInput")
    iota17 = nc.dram_tensor("iota17", [1, CA], F32, kind="ExternalInput")

    score_o = nc.dram_tensor("score_o", [BSH, 1], F32, kind="ExternalOutput")
    tag_o = nc.dram_tensor("tag_o", [BSH, 1], F32, kind="ExternalOutput")
    bps_o = nc.dram_tensor("bps_o", [128, NCH * 8 * CA], U8,
                           kind="ExternalOutput")

    from contextlib import ExitStack
    with tile.TileContext(nc) as tc, ExitStack() as ctx:
        consts = ctx.enter_context(tc.tile_pool(name="consts", bufs=1))
        sbig = ctx.enter_context(tc.tile_pool(name="sbig", bufs=1))
        featp = ctx.enter_context(tc.tile_pool(name="featp", bufs=3))
        ftt = ctx.enter_context(tc.tile_pool(name="ftt", bufs=2))
        esbp = ctx.enter_context(tc.tile_pool(name="esbp", bufs=3))
        emisp = ctx.enter_context(tc.tile_pool(name="emisp", bufs=6))
        accp = ctx.enter_context(tc.tile_pool(name="accp", bufs=2))
        repp = ctx.enter_context(tc.tile_pool(name="repp", bufs=3))
        sxp = ctx.enter_context(tc.tile_pool(name="sxp", bufs=3))
        bpp = ctx.enter_context(tc.tile_pool(name="bpp", bufs=3))
        finp = ctx.enter_context(tc.tile_pool(name="finp", bufs=1))
        psp = ctx.enter_context(tc.tile_pool(name="psp", bufs=2, space="PSUM"))
        pse = ctx.enter_context(tc.tile_pool(name="pse", bufs=4, space="PSUM"))
        if True:
            # ---- constants -------------------------------------------------
            ident = consts.tile([128, 128], F32)
            make_identity(nc, ident)
            w_sb = consts.tile([128, 6, CA], F32)
            nc.sync.dma_start(out=w_sb,
                              in_=wt[:, :].rearrange("(k p) c -> p k c", p=128))

            def bcast_const(dram, n_part, shape):
                t = consts.tile([n_part] + shape, F32, name=f"c_{dram.name}_{n_part}")
                nc.sync.dma_start(
                    out=t, in_=bass.AP(tensor=dram, offset=0,
                                       ap=[[0, n_part], [1, int(np.prod(shape))]]))
                return t

            trep32 = bcast_const(trep, BSH, [CC])
            trepx128 = bcast_const(trepx, 128, [8 * CC])
            iotamx128 = bcast_const(iotamx, 128, [8 * CC])
            tcol32 = bcast_const(tcol, BSH, [CA])
            tstop32 = bcast_const(tstop, BSH, [CA])
            iota17_32 = bcast_const(iota17, BSH, [CA])
            bfc_sb = consts.tile([1, CA], F32)
            nc.sync.dma_start(out=bfc_sb, in_=bfc[:, :])
            ones1 = consts.tile([1, 128], F32)
            nc.vector.memset(ones1, 1.0)

            # ---- state buffers --------------------------------------------
            # s_ch[i] holds scores s_t for t in [32i, 32i+32)
            # maxv_ch[i] holds pre-emis maxes for t in [32i+1, 32i+33)
            s_ch = [sbig.tile([BSH, 32, CA], F32, name=f"s{i}", tag=f"s{i}")
                    for i in range(NCH)]
            maxv_ch = [sbig.tile([BSH, 32, CA], F32, name=f"m{i}", tag=f"m{i}")
                       for i in range(NCH)]
            bps_f = sbig.tile([128, NCH, 8, CA], F32)
            bps_u8 = sbig.tile([128, NCH * 8 * CA], U8)
            # slot for t=512 (never produced; bp output there is discarded)
            nc.vector.memset(maxv_ch[NCH - 1][:, 31, :], 0.0)

            emis_ch = {}

            # ---- phase E: emis = feat @ W (+ bias), relayout ---------------
            # One E-tile covers 4 sequences x 32 timesteps (rows (u, t32)),
            # so emis chunk i is complete after its own 8 tiles. The PE part
            # is emitted two chunks ahead of the forward steps, the DVE
            # PSUM->SBUF copies + scatters one chunk ahead, so the in-order
            # DVE queue never waits on the PE.
            eps_pend = {}

            def emit_epe(i):
                # E-tile (gp, gh) covers sequences b(u) = 8gp + 2u + gh on
                # rows (u, t32); this b-striping lets one stride-2 partition
                # DMA scatter each gh half into the emis chunk tile.
                eps_list = []
                for gp in range(4):          # pairs of groups of 4 sequences
                    eps = pse.tile([128, 2, CA], F32, tag="eps")
                    for gh in range(2):
                        feat_sb = featp.tile([128, D], F32, tag="feat")
                        nc.sync.dma_start(
                            out=feat_sb,
                            in_=bass.AP(
                                tensor=feat,
                                offset=((8 * gp + gh) * L + 32 * i) * D,
                                ap=[[2 * L * D, 4], [D, 32], [1, D]]))
                        featT = ftt.tile([128, 6, 128], F32, tag="ftT")
                        for h in range(2):
                            ftps = psp.tile([128, 3, 128], F32, tag="ftps")
                            for k3 in range(3):
                                k = 3 * h + k3
                                nc.tensor.transpose(
                                    ftps[:, k3, :],
                                    feat_sb[:, k * 128:(k + 1) * 128], ident)
                            nc.scalar.copy(out=featT[:, 3 * h:3 * h + 3, :],
                                           in_=ftps)
                        for k in range(6):
                            nc.tensor.matmul(eps[:, gh, :], lhsT=featT[:, k, :],
                                             rhs=w_sb[:, k, :],
                                             start=(k == 0), stop=False)
                        # bias as a K=1 accumulation: ones.T @ b_fc
                        nc.tensor.matmul(eps[:, gh, :], lhsT=ones1, rhs=bfc_sb,
                                         start=False, stop=True)
                    eps_list.append(eps)
                eps_pend[i] = eps_list

            def emit_ecopy(i):
                emis_ch[i] = emisp.tile([BSH, 32, CA], F32,
                                        name=f"emis{i}", tag="emis")
                ebase = emis_ch[i][:, :, :]
                pstride = ebase.ap[0][0]
                for gp, eps in enumerate(eps_pend.pop(i)):
                    emis_sb = esbp.tile([128, 2, CA], F32, tag="esb")
                    nc.vector.tensor_copy(out=emis_sb, in_=eps)
                    for gh in range(2):
                        # rows b = 8gp + 2u + gh (u = 0..3), free (t32, c)
                        dst = bass.AP(
                            tensor=ebase.tensor,
                            offset=ebase.offset + (8 * gp + gh) * pstride,
                            ap=[[2 * pstride, 4], [CA, 32], [1, CA]])
                        nc.scalar.dma_start(out=dst,
                                            in_=emis_sb[:, gh, :])

            # ---- phase F: forward recurrence (DVE, serial) -----------------
            def fwd_step(t):
                if t == 0:
                    nc.vector.tensor_tensor(
                        out=s_ch[0][:, 0, :], in0=tcol32,
                        in1=emis_ch[0][:, 0, :], op=mybir.AluOpType.add)
                    return
                i, r = divmod(t, 32)
                im, rm = divmod(t - 1, 32)
                prev = s_ch[im][:, rm, :]
                acc = accp.tile([BSH, CC], F32, tag="acc")
                nc.vector.tensor_tensor(
                    out=acc, in0=_ap(prev, [[0, CA], [1, CA]]), in1=trep32,
                    op=mybir.AluOpType.add)
                nc.vector.tensor_reduce(
                    out=maxv_ch[im][:, rm, :],
                    in_=acc.rearrange("p (c k) -> p c k", c=CA),
                    axis=mybir.AxisListType.X, op=mybir.AluOpType.max)
                nc.vector.tensor_tensor(
                    out=s_ch[i][:, r, :], in0=maxv_ch[im][:, rm, :],
                    in1=emis_ch[i][:, r, :], op=mybir.AluOpType.add)

            # ---- phase BP: backpointer recovery for chunk i ----------------
            # bp chunk i covers t = 32i + 4j + q + 1, j in 0..7, q in 0..3,
            # laid out on partition 4b+q, free (j, c).
            def rep_dma(dst, src_ch):
                # dst row 4b+q, free (j, c)  <-  src[b, 4j+q, c]
                base = src_ch[:, :, :]
                for q in range(4):
                    dst_q = bass.AP(
                        tensor=dst.tensor,
                        offset=dst.offset + q * dst.ap[0][0],
                        ap=[[4 * dst.ap[0][0], 32], [CA, 8], [1, CA]])
                    src_q = bass.AP(
                        tensor=base.tensor, offset=base.offset + q * CA,
                        ap=[list(base.ap[0]), [4 * CA, 8], [1, CA]])
                    nc.scalar.dma_start(out=dst_q, in_=src_q)

            # bp pass, 3 stages pipelined behind the forward so the
            # in-order DVE queue never waits on DMA or GPSIMD results.
            # acc2 layout is (c, j, c'): the s-expansion is then a single
            # 3-dim block-broadcast DMA; m stays a broadcast AP on the DVE
            # is_equal; GPSIMD runs the two plain elementwise ops.
            bp_st = {}

            def bp_stage1(i):
                s_rep = repp.tile([128, 8, CA], F32, tag="srep")
                rep_dma(s_rep, s_ch[i])
                m_rep = repp.tile([128, 8, CA], F32, tag="mrep")
                rep_dma(m_rep, maxv_ch[i])
                s_exp = sxp.tile([128, 8 * CC], F32, tag="sexp")
                nc.scalar.dma_start(
                    out=s_exp,
                    in_=_ap(s_rep, [[0, CA], [1, 8 * CA]]))
                acc2 = bpp.tile([128, 8 * CC], F32, tag="acc2")
                nc.gpsimd.tensor_tensor(out=acc2, in0=s_exp, in1=trepx128,
                                        op=mybir.AluOpType.add)
                bp_st[i] = (m_rep, acc2)

            def bp_stage2(i):
                m_rep, acc2 = bp_st[i]
                eqv = sxp.tile([128, 8 * CC], F32, tag="sexp", name="eqv")
                nc.vector.tensor_tensor(
                    out=eqv, in0=acc2,
                    in1=_ap(m_rep, [[1, CA], [CA, 8], [0, CA]]),
                    op=mybir.AluOpType.is_equal)
                nc.gpsimd.tensor_tensor(out=acc2, in0=eqv, in1=iotamx128,
                                        op=mybir.AluOpType.mult)

            def bp_stage3(i):
                _, acc2 = bp_st.pop(i)
                nc.vector.tensor_reduce(
                    out=_ap(bps_f[:, i, :, :], [[1, CA], [CA, 8]]),
                    in_=acc2.rearrange("p (c j k) -> p c j k", c=CA, j=8),
                    axis=mybir.AxisListType.X, op=mybir.AluOpType.min)

            # Emission order: PE part two chunks ahead, copies one ahead,
            # bp stages trail the forward by 1/2/3 chunks.
            if skip_emis:
                def emit_epe(i):  # noqa: F811
                    pass

                def emit_ecopy(i):  # noqa: F811
                    emis_ch[i] = emisp.tile([BSH, 32, CA], F32,
                                            name=f"emis{i}", tag="emis")
                    nc.gpsimd.memset(emis_ch[i], 0.1)
            if skip_bp:
                def bp_stage1(i):  # noqa: F811
                    pass

                def bp_stage2(i):  # noqa: F811
                    pass

                def bp_stage3(i):  # noqa: F811
                    pass
            emit_epe(0)
            emit_epe(1)
            emit_ecopy(0)
            for i in range(NCH):
                if i + 2 < NCH:
                    emit_epe(i + 2)
                if i + 1 < NCH:
                    emit_ecopy(i + 1)
                for t in range(32 * i, 32 * i + 32):
                    fwd_step(t)
                if i >= 1:
                    bp_stage1(i - 1)
                if i >= 2:
                    bp_stage2(i - 2)
                if i >= 3:
                    bp_stage3(i - 3)
            bp_stage1(NCH - 1)
            bp_stage2(NCH - 2)
            bp_stage3(NCH - 3)
            bp_stage2(NCH - 1)
            bp_stage3(NCH - 2)
            bp_stage3(NCH - 1)

            # ---- final: best score / tag ----------------------------------
            fin = finp.tile([BSH, CA], F32)
            nc.vector.tensor_tensor(out=fin, in0=s_ch[NCH - 1][:, 31, :],
                                    in1=tstop32, op=mybir.AluOpType.add)
            best = finp.tile([BSH, 1], F32)
            nc.vector.tensor_reduce(out=best, in_=fin,
                                    axis=mybir.AxisListType.X,
                                    op=mybir.AluOpType.max)
            eqf = finp.tile([BSH, CA], F32)
            nc.vector.tensor_tensor(out=eqf, in0=fin,
                                    in1=_ap(best, [[0, CA]]),
                                    op=mybir.AluOpType.is_equal)
            nc.vector.tensor_tensor(out=eqf, in0=eqf, in1=iota17_32,
                                    op=mybir.AluOpType.mult)
            tagf = finp.tile([BSH, 1], F32)
            nc.vector.tensor_reduce(out=tagf, in_=eqf,
                                    axis=mybir.AxisListType.X,
                                    op=mybir.AluOpType.min)
            nc.sync.dma_start(out=score_o[:, :], in_=best)
            nc.sync.dma_start(out=tag_o[:, :], in_=tagf)

            # ---- bp table to u8, out --------------------------------------
            if skip_bp:
                nc.vector.memset(bps_f, 0.0)
            nc.vector.tensor_scalar(
                out=bps_u8, in0=bps_f.rearrange("p a b c -> p (a b c)"),
                scalar1=BIG, scalar2=None, op0=mybir.AluOpType.add)
            nc.sync.dma_start(out=bps_o[:, :], in_=bps_u8)

    nc.compile()
    return nc


_NC_CACHE = {}


def _get_nc():
    if "nc" not in _NC_CACHE:
        _NC_CACHE["nc"] = build_bass()
    return _NC_CACHE["nc"]


def kernel(features, masks, W_fc, b_fc, transitions):
    features = np.ascontiguousarray(np.asarray(features, dtype=np.float32))
    masks = np.asarray(masks, dtype=np.float32)
    W_fc = np.asarray(W_fc, dtype=np.float32)
    b_fc = np.asarray(b_fc, dtype=np.float32)
    transitions = np.asarray(transitions, dtype=np.float32)

    if not np.all(masks == 1.0):
        # Suffix padding would be handled by the masked backtrack below plus
        # per-step predication; the benchmark always supplies all-ones masks.
        raise NotImplementedError("only all-ones masks are supported")

    wt = np.ascontiguousarray(W_fc[:CA, :].T)                    # [D, CA]
    T_act = np.ascontiguousarray(transitions[:CA, :CA])
    trep = T_act.reshape(1, CC)
    # (c, j, c') order for the bp pass
    trepx = np.repeat(T_act[:, None, :], 8, axis=1).reshape(1, 8 * CC)
    iotamx = np.tile(np.arange(CA, dtype=np.float32) - BIG, CA * 8).reshape(1, 8 * CC)
    tcol = np.ascontiguousarray(transitions[:CA, START]).reshape(1, CA)
    tstop = np.ascontiguousarray(transitions[STOP, :CA]).reshape(1, CA)
    bfc = b_fc[:CA].reshape(1, CA)
    iota17 = (np.arange(CA, dtype=np.float32) - BIG).reshape(1, CA)

    nc = _get_nc()
    in_maps = []
    for k in range(N_CORES):
        fshard = features[k * BSH:(k + 1) * BSH].reshape(R, D)
        in_maps.append({"feat": fshard, "wt": wt, "trep": trep,
                        "trepx": trepx, "iotamx": iotamx, "tcol": tcol,
                        "tstop": tstop, "bfc": bfc, "iota17": iota17})
    res = run_bass_kernel_spmd(nc, in_maps, core_ids=list(range(N_CORES)))

    best_score = np.empty([B], np.float32)
    best_tag = np.empty([B], np.int64)
    bp = np.empty([B, L, CA], np.uint8)   # bp[:, t] valid for t in 1..511
    for k in range(N_CORES):
        r = res.results[k]
        sl = slice(k * BSH, (k + 1) * BSH)
        best_score[sl] = r["score_o"][:, 0]
        best_tag[sl] = (r["tag_o"][:, 0] + BIG).astype(np.int64)
        # row 4b+q, free (i, j, c) -> t-1 = 32i + 4j + q
        arr = r["bps_o"].reshape(BSH, 4, NCH, 8, CA)      # (b, q, i, j, c)
        arr = arr.transpose(0, 2, 3, 1, 4).reshape(BSH, L, CA)  # t-1 major
        bp[sl, 1:, :] = arr[:, :L - 1, :]

    # host backtrack (t = L-1 .. 0); masks are all ones here
    tags = np.empty([B, L], np.int32)
    cur = best_tag.copy()
    bidx = np.arange(B)
    for t in range(L - 1, -1, -1):
        tags[:, t] = cur
        if t > 0:
            cur = bp[bidx, t, cur].astype(np.int64)
    return best_score, tags


# revision 31
# speedup vs baseline: 1.0010x; 1.0010x over previous
"""CRF Viterbi decode kernel for Trainium2 (8 NeuronCores, Bass/Tile).

Problem: B=256, L=512, D=768, C=19 (17 tags + START + STOP).
  emis = features @ W_fc.T + b_fc                        [B, L, C]
  Viterbi forward (max-plus recurrence over L) + backpointers
  -> best_score [B] f32, best_paths [B, L] int32

Sharding: data-parallel over batch B -> 32 sequences per core, batch on
SBUF partitions. Small weights (W_fc, transitions) replicated.

Exactness strategy: the reference's argmax decisions are reproduced
bit-for-bit. The DP is restricted to the 17 "active" tags: the START row
and STOP column of `transitions` are -10000, so START/STOP never win any
argmax after t=0 (scores stay within a few hundred of each other while
the -10000 offset is insurmountable); t=0 is handled in closed form
(s_0 = T[:, START] + emis_0). All forward adds/maxes use the same fp32
operation order as the reference. Backpointers are recovered by an
equality pass against the stored per-step max values (recomputing
prev + T with the identical fp32 add), which reproduces jnp.argmax's
first-max-index semantics exactly, ties included.

Per-core phases (engine):
  E  (DMA/PE/ACT): stream features, PE-transpose 128x128 blocks,
      matmul vs W (+bias as a K=1 matmul) -> emis, scatter into
      [32b, 32t, 17c] chunk tiles; PE part runs two chunks ahead of the
      forward, ACT PSUM->SBUF copies one chunk ahead
  F  (DVE, serial): 511 x (add T -> segmented max -> add emis), the
      critical path (~500 us of DVE busy in the cost model)
  BP (3 stages trailing F by 1/2/3 chunks): DMA replicates scores 4x
      across partitions + block-broadcasts them; GPSIMD adds T and
      multiplies the iota mask; DVE does the exact is_equal against the
      stored per-step maxes and the first-max-index min-reduce
  BT (host): vectorized numpy backtrack over the downloaded bp table

Under this axon client no NTFF/neuron-profile path exists; per the
TimelineSim instruction cost model one core executes in ~727 us
(DVE-bound), overlapping the ~50 MB/core feature stream, PE matmuls,
GPSIMD bp work and the DMA traffic.
"""

import numpy as np

import concourse.bass as bass
import concourse.mybir as mybir
import concourse.tile as tile
from concourse import bacc
from concourse.bass_utils import run_bass_kernel_spmd
from concourse.masks import make_identity

F32 = mybir.dt.float32
U8 = mybir.dt.uint8

N_CORES = 8
B, L, D, C = 256, 512, 768, 19
CA = 17                      # active tags (START=17, STOP=18 excluded)
CC = CA * CA                 # 289
BSH = B // N_CORES           # 32 sequences per core
R = BSH * L                  # 16384 rows of the emis matmul per core
NCH = L // 32                # 16 chunks of 32 timesteps
START, STOP = C - 2, C - 1
BIG = 1.0e6


def _ap(base, pairs):
    """AP with the partition pair of `base` and explicit free pairs."""
    return bass.AP(tensor=base.tensor, offset=base.offset,
                   ap=[list(base.ap[0])] + [list(p) for p in pairs])


def build_bass(skip_bp=False, skip_emis=False):
    nc = bacc.Bacc("TRN2", target_bir_lowering=False, debug=False,
                   num_devices=N_CORES)

    feat = nc.dram_tensor("feat", [R, D], F32, kind="ExternalInput")
    wt = nc.dram_tensor("wt", [D, CA], F32, kind="ExternalInput")
    trep = nc.dram_tensor("trep", [1, CC], F32, kind="ExternalInput")
    trepx = nc.dram_tensor("trepx", [1, 8 * CC], F32, kind="ExternalInput")
    iotamx = nc.dram_tensor("iotamx", [1, 8 * CC], F32, kind="ExternalInput")
    tcol = nc.dram_tensor("tcol", [1, CA], F32, kind="ExternalInput")
    tstop = nc.dram_tensor("tstop", [1, CA], F32, kind="ExternalInput")
    bfc = nc.dram_tensor("bfc", [1, CA], F32, kind="ExternalInput")
    iota17 = nc.dram_tensor("iota17", [1, CA], F32, kind="ExternalInput")

    score_o = nc.dram_tensor("score_o", [BSH, 1], F32, kind="ExternalOutput")
    tag_o = nc.dram_tensor("tag_o", [BSH, 1], F32, kind="ExternalOutput")
    bps_o = nc.dram_tensor("bps_o", [128, NCH * 8 * CA], U8,
                           kind="ExternalOutput")

    from contextlib import ExitStack
    with tile.TileContext(nc) as tc, ExitStack() as ctx:
        consts = ctx.enter_context(tc.tile_pool(name="consts", bufs=1))
        sbig = ctx.enter_context(tc.tile_pool(name="sbig", bufs=1))
        featp = ctx.enter_context(tc.tile_pool(name="featp", bufs=3))
        ftt = ctx.enter_context(tc.tile_pool(name="ftt", bufs=2))
        esbp = ctx.enter_context(tc.tile_pool(name="esbp", bufs=3))
        emisp = ctx.enter_context(tc.tile_pool(name="emisp", bufs=8))
        accp = ctx.enter_context(tc.tile_pool(name="accp", bufs=3))
        repp = ctx.enter_context(tc.tile_pool(name="repp", bufs=3))
        sxp = ctx.enter_context(tc.tile_pool(name="sxp", bufs=3))
        bpp = ctx.enter_context(tc.tile_pool(name="bpp", bufs=3))
        finp = ctx.enter_context(tc.tile_pool(name="finp", bufs=1))
        psp = ctx.enter_context(tc.tile_pool(name="psp", bufs=2, space="PSUM"))
        pse = ctx.enter_context(tc.tile_pool(name="pse", bufs=6, space="PSUM"))
        if True:
            # ---- constants -------------------------------------------------
            ident = consts.tile([128, 128], F32)
            make_identity(nc, ident)
            w_sb = consts.tile([128, 6, CA], F32)
            nc.sync.dma_start(out=w_sb,
                              in_=wt[:, :].rearrange("(k p) c -> p k c", p=128))

            def bcast_const(dram, n_part, shape):
                t = consts.tile([n_part] + shape, F32, name=f"c_{dram.name}_{n_part}")
                nc.sync.dma_start(
                    out=t, in_=bass.AP(tensor=dram, offset=0,
                                       ap=[[0, n_part], [1, int(np.prod(shape))]]))
                return t

            trep32 = bcast_const(trep, BSH, [CC])
            trepx128 = bcast_const(trepx, 128, [8 * CC])
            iotamx128 = bcast_const(iotamx, 128, [8 * CC])
            tcol32 = bcast_const(tcol, BSH, [CA])
            tstop32 = bcast_const(tstop, BSH, [CA])
            iota17_32 = bcast_const(iota17, BSH, [CA])
            bfc_sb = consts.tile([1, CA], F32)
            nc.sync.dma_start(out=bfc_sb, in_=bfc[:, :])
            ones1 = consts.tile([1, 128], F32)
            nc.vector.memset(ones1, 1.0)

            # ---- state buffers --------------------------------------------
            # s_ch[i] holds scores s_t for t in [32i, 32i+32)
            # maxv_ch[i] holds pre-emis maxes for t in [32i+1, 32i+33)
            s_ch = [sbig.tile([BSH, 32, CA], F32, name=f"s{i}", tag=f"s{i}")
                    for i in range(NCH)]
            maxv_ch = [sbig.tile([BSH, 32, CA], F32, name=f"m{i}", tag=f"m{i}")
                       for i in range(NCH)]
            bps_f = sbig.tile([128, NCH, 8, CA], F32)
            bps_u8 = sbig.tile([128, NCH * 8 * CA], U8)
            # slot for t=512 (never produced; bp output there is discarded)
            nc.vector.memset(maxv_ch[NCH - 1][:, 31, :], 0.0)

            emis_ch = {}

            # ---- phase E: emis = feat @ W (+ bias), relayout ---------------
            # One E-tile covers 4 sequences x 32 timesteps (rows (u, t32)),
            # so emis chunk i is complete after its own 8 tiles. The PE part
            # is emitted two chunks ahead of the forward steps, the DVE
            # PSUM->SBUF copies + scatters one chunk ahead, so the in-order
            # DVE queue never waits on the PE.
            eps_pend = {}

            def emit_epe(i):
                # E-tile (gp, gh) covers sequences b(u) = 8gp + 2u + gh on
                # rows (u, t32); this b-striping lets one stride-2 partition
                # DMA scatter each gh half into the emis chunk tile.
                eps_list = []
                for gp in range(4):          # pairs of groups of 4 sequences
                    eps = pse.tile([128, 2, CA], F32, tag="eps")
                    for gh in range(2):
                        feat_sb = featp.tile([128, D], F32, tag="feat")
                        nc.sync.dma_start(
                            out=feat_sb,
                            in_=bass.AP(
                                tensor=feat,
                                offset=((8 * gp + gh) * L + 32 * i) * D,
                                ap=[[2 * L * D, 4], [D, 32], [1, D]]))
                        featT = ftt.tile([128, 6, 128], F32, tag="ftT")
                        for h in range(2):
                            ftps = psp.tile([128, 3, 128], F32, tag="ftps")
                            for k3 in range(3):
                                k = 3 * h + k3
                                nc.tensor.transpose(
                                    ftps[:, k3, :],
                                    feat_sb[:, k * 128:(k + 1) * 128], ident)
                            nc.scalar.copy(out=featT[:, 3 * h:3 * h + 3, :],
                                           in_=ftps)
                        for k in range(6):
                            nc.tensor.matmul(eps[:, gh, :], lhsT=featT[:, k, :],
                                             rhs=w_sb[:, k, :],
                                             start=(k == 0), stop=False)
                        # bias as a K=1 accumulation: ones.T @ b_fc
                        nc.tensor.matmul(eps[:, gh, :], lhsT=ones1, rhs=bfc_sb,
                                         start=False, stop=True)
                    eps_list.append(eps)
                eps_pend[i] = eps_list

            def emit_ecopy(i):
                emis_ch[i] = emisp.tile([BSH, 32, CA], F32,
                                        name=f"emis{i}", tag="emis")
                ebase = emis_ch[i][:, :, :]
                pstride = ebase.ap[0][0]
                for gp, eps in enumerate(eps_pend.pop(i)):
                    emis_sb = esbp.tile([128, 2, CA], F32, tag="esb")
                    nc.scalar.copy(out=emis_sb, in_=eps)
                    for gh in range(2):
                        # rows b = 8gp + 2u + gh (u = 0..3), free (t32, c)
                        dst = bass.AP(
                            tensor=ebase.tensor,
                            offset=ebase.offset + (8 * gp + gh) * pstride,
                            ap=[[2 * pstride, 4], [CA, 32], [1, CA]])
                        nc.scalar.dma_start(out=dst,
                                            in_=emis_sb[:, gh, :])

            # ---- phase F: forward recurrence (DVE, serial) -----------------
            def fwd_step(t):
                if t == 0:
                    nc.vector.tensor_tensor(
                        out=s_ch[0][:, 0, :], in0=tcol32,
                        in1=emis_ch[0][:, 0, :], op=mybir.AluOpType.add)
                    return
                i, r = divmod(t, 32)
                im, rm = divmod(t - 1, 32)
                prev = s_ch[im][:, rm, :]
                acc = accp.tile([BSH, CC], F32, tag="acc")
                nc.vector.tensor_tensor(
                    out=acc, in0=_ap(prev, [[0, CA], [1, CA]]), in1=trep32,
                    op=mybir.AluOpType.add)
                nc.vector.tensor_reduce(
                    out=maxv_ch[im][:, rm, :],
                    in_=acc.rearrange("p (c k) -> p c k", c=CA),
                    axis=mybir.AxisListType.X, op=mybir.AluOpType.max)
                nc.vector.tensor_tensor(
                    out=s_ch[i][:, r, :], in0=maxv_ch[im][:, rm, :],
                    in1=emis_ch[i][:, r, :], op=mybir.AluOpType.add)

            # ---- phase BP: backpointer recovery for chunk i ----------------
            # bp chunk i covers t = 32i + 4j + q + 1, j in 0..7, q in 0..3,
            # laid out on partition 4b+q, free (j, c).
            def rep_dma(dst, src_ch):
                # dst row 4b+q, free (j, c)  <-  src[b, 4j+q, c]
                base = src_ch[:, :, :]
                for q in range(4):
                    dst_q = bass.AP(
                        tensor=dst.tensor,
                        offset=dst.offset + q * dst.ap[0][0],
                        ap=[[4 * dst.ap[0][0], 32], [CA, 8], [1, CA]])
                    src_q = bass.AP(
                        tensor=base.tensor, offset=base.offset + q * CA,
                        ap=[list(base.ap[0]), [4 * CA, 8], [1, CA]])
                    nc.scalar.dma_start(out=dst_q, in_=src_q)

            # bp pass, 3 stages pipelined behind the forward so the
            # in-order DVE queue never waits on DMA or GPSIMD results.
            # acc2 layout is (c, j, c'): the s-expansion is then a single
            # 3-dim block-broadcast DMA; m stays a broadcast AP on the DVE
            # is_equal; GPSIMD runs the two plain elementwise ops.
            bp_st = {}

            def bp_stage1(i):
                s_rep = repp.tile([128, 8, CA], F32, tag="srep")
                rep_dma(s_rep, s_ch[i])
                m_rep = repp.tile([128, 8, CA], F32, tag="mrep")
                rep_dma(m_rep, maxv_ch[i])
                s_exp = sxp.tile([128, 8 * CC], F32, tag="sexp")
                nc.scalar.dma_start(
                    out=s_exp,
                    in_=_ap(s_rep, [[0, CA], [1, 8 * CA]]))
                acc2 = bpp.tile([128, 8 * CC], F32, tag="acc2")
                nc.gpsimd.tensor_tensor(out=acc2, in0=s_exp, in1=trepx128,
                                        op=mybir.AluOpType.add)
                bp_st[i] = (m_rep, acc2)

            def bp_stage2(i):
                m_rep, acc2 = bp_st[i]
                eqv = sxp.tile([128, 8 * CC], F32, tag="sexp", name="eqv")
                nc.vector.tensor_tensor(
                    out=eqv, in0=acc2,
                    in1=_ap(m_rep, [[1, CA], [CA, 8], [0, CA]]),
                    op=mybir.AluOpType.is_equal)
                nc.gpsimd.tensor_tensor(out=acc2, in0=eqv, in1=iotamx128,
                                        op=mybir.AluOpType.mult)

            def bp_stage3(i):
                _, acc2 = bp_st.pop(i)
                nc.vector.tensor_reduce(
                    out=_ap(bps_f[:, i, :, :], [[1, CA], [CA, 8]]),
                    in_=acc2.rearrange("p (c j k) -> p c j k", c=CA, j=8),
                    axis=mybir.AxisListType.X, op=mybir.AluOpType.min)

            # Emission order: PE part two chunks ahead, copies one ahead,
            # bp stages trail the forward by 1/2/3 chunks.
            if skip_emis:
                def emit_epe(i):  # noqa: F811
                    pass

                def emit_ecopy(i):  # noqa: F811
                    emis_ch[i] = emisp.tile([BSH, 32, CA], F32,
                                            name=f"emis{i}", tag="emis")
                    nc.gpsimd.memset(emis_ch[i], 0.1)
            if skip_bp:
                def bp_stage1(i):  # noqa: F811
                    pass

                def bp_stage2(i):  # noqa: F811
                    pass

                def bp_stage3(i):  # noqa: F811
                    pass
            emit_epe(0)
            emit_epe(1)
            emit_ecopy(0)
            for i in range(NCH):
                if i + 2 < NCH:
                    emit_epe(i + 2)
                if i + 1 < NCH:
                    emit_ecopy(i + 1)
                for t in range(32 * i, 32 * i + 32):
                    fwd_step(t)
                if i >= 1:
                    bp_stage1(i - 1)
                if i >= 2:
                    bp_stage2(i - 2)
                if i >= 3:
                    bp_stage3(i - 3)
            bp_stage1(NCH - 1)
            bp_stage2(NCH - 2)
            bp_stage3(NCH - 3)
            bp_stage2(NCH - 1)
            bp_stage3(NCH - 2)
            bp_stage3(NCH - 1)

            # ---- final: best score / tag ----------------------------------
            fin = finp.tile([BSH, CA], F32)
            nc.vector.tensor_tensor(out=fin, in0=s_ch[NCH - 1][:, 31, :],
                                    in1=tstop32, op=mybir.AluOpType.add)
            best = finp.tile([BSH, 1], F32)
            nc.vector.tensor_reduce(out=best, in_=fin,
                                    axis=mybir.AxisListType.X,
                                    op=mybir.AluOpType.max)
            eqf = finp.tile([BSH, CA], F32)
            nc.vector.tensor_tensor(out=eqf, in0=fin,
                                    in1=_ap(best, [[0, CA]]),
                                    op=mybir.AluOpType.is_equal)
            nc.vector.tensor_tensor(out=eqf, in0=eqf, in1=iota17_32,
                                    op=mybir.AluOpType.mult)
            tagf = finp.tile([BSH, 1], F32)
            nc.vector.tensor_reduce(out=tagf, in_=eqf,
                                    axis=mybir.AxisListType.X,
                                    op=mybir.AluOpType.min)
            nc.sync.dma_start(out=score_o[:, :], in_=best)
            nc.sync.dma_start(out=tag_o[:, :], in_=tagf)

            # ---- bp table to u8, out --------------------------------------
            if skip_bp:
                nc.vector.memset(bps_f, 0.0)
            nc.vector.tensor_scalar(
                out=bps_u8, in0=bps_f.rearrange("p a b c -> p (a b c)"),
                scalar1=BIG, scalar2=None, op0=mybir.AluOpType.add)
            nc.sync.dma_start(out=bps_o[:, :], in_=bps_u8)

    nc.compile()
    return nc


_NC_CACHE = {}


def _get_nc():
    if "nc" not in _NC_CACHE:
        _NC_CACHE["nc"] = build_bass()
    return _NC_CACHE["nc"]


def kernel(features, masks, W_fc, b_fc, transitions):
    features = np.ascontiguousarray(np.asarray(features, dtype=np.float32))
    masks = np.asarray(masks, dtype=np.float32)
    W_fc = np.asarray(W_fc, dtype=np.float32)
    b_fc = np.asarray(b_fc, dtype=np.float32)
    transitions = np.asarray(transitions, dtype=np.float32)

    if not np.all(masks == 1.0):
        # Suffix padding would be handled by the masked backtrack below plus
        # per-step predication; the benchmark always supplies all-ones masks.
        raise NotImplementedError("only all-ones masks are supported")

    wt = np.ascontiguousarray(W_fc[:CA, :].T)                    # [D, CA]
    T_act = np.ascontiguousarray(transitions[:CA, :CA])
    trep = T_act.reshape(1, CC)
    # (c, j, c') order for the bp pass
    trepx = np.repeat(T_act[:, None, :], 8, axis=1).reshape(1, 8 * CC)
    iotamx = np.tile(np.arange(CA, dtype=np.float32) - BIG, CA * 8).reshape(1, 8 * CC)
    tcol = np.ascontiguousarray(transitions[:CA, START]).reshape(1, CA)
    tstop = np.ascontiguousarray(transitions[STOP, :CA]).reshape(1, CA)
    bfc = b_fc[:CA].reshape(1, CA)
    iota17 = (np.arange(CA, dtype=np.float32) - BIG).reshape(1, CA)

    nc = _get_nc()
    in_maps = []
    for k in range(N_CORES):
        fshard = features[k * BSH:(k + 1) * BSH].reshape(R, D)
        in_maps.append({"feat": fshard, "wt": wt, "trep": trep,
                        "trepx": trepx, "iotamx": iotamx, "tcol": tcol,
                        "tstop": tstop, "bfc": bfc, "iota17": iota17})
    res = run_bass_kernel_spmd(nc, in_maps, core_ids=list(range(N_CORES)))

    best_score = np.empty([B], np.float32)
    best_tag = np.empty([B], np.int64)
    bp = np.empty([B, L, CA], np.uint8)   # bp[:, t] valid for t in 1..511
    for k in range(N_CORES):
        r = res.results[k]
        sl = slice(k * BSH, (k + 1) * BSH)
        best_score[sl] = r["score_o"][:, 0]
        best_tag[sl] = (r["tag_o"][:, 0] + BIG).astype(np.int64)
        # row 4b+q, free (i, j, c) -> t-1 = 32i + 4j + q
        arr = r["bps_o"].reshape(BSH, 4, NCH, 8, CA)      # (b, q, i, j, c)
        arr = arr.transpose(0, 2, 3, 1, 4).reshape(BSH, L, CA)  # t-1 major
        bp[sl, 1:, :] = arr[:, :L - 1, :]

    # host backtrack (t = L-1 .. 0); masks are all ones here
    tags = np.empty([B, L], np.int32)
    cur = best_tag.copy()
    bidx = np.arange(B)
    for t in range(L - 1, -1, -1):
        tags[:, t] = cur
        if t > 0:
            cur = bp[bidx, t, cur].astype(np.int64)
    return best_score, tags
